# revision 12
# baseline (speedup 1.0000x reference)
"""Bahdanau-style attention kernel for Trainium2, data-parallel over batch.

Math (per (s, b)):
    pre[s,b,:]  = We @ enc[s,b,:] + Wh @ hidden[b,:] + attn_b      (H outputs)
    energies    = score_w . tanh(pre)                               -> [S, B]
    out         = softmax over S of (energies masked to -1e12)      -> [B, 1, S]

Sharding: B=16 batches split 2-per-core over 8 NeuronCores; weights are
replicated; no collectives. Each core runs one identical Bass program on
its own input slice.

Per-core layout: the main GEMM computes proj^T tiles [h_out=128, s=512]
with We^T chunks stationary (fp32r -> full PE rate at free dim 512) and
enc^T tiles moving; tanh+bias (attn_b + Wh@hidden, both per-partition
constants for a fixed b) is fused into one ScalarE activation reading
PSUM; the score contraction over h_out is a second accumulating matmul
(score column stationary), so energies for 512 positions land in PSUM as
[1, 512]; masking adds -1e12*mask and exp+sum are fused via activation
accum_out (max-subtraction is skipped: energies are O(1) bounded by
|score_w|_1, so exp never overflows, and exp(-1e12) == 0 exactly).
"""

import sys

for _p in ("/opt/trn_rl_repo", "/opt/pypackages"):
    if _p not in sys.path:
        sys.path.append(_p)

import numpy as np

from concourse import bacc, mybir, tile
from concourse.bass_utils import run_bass_kernel_spmd

H = 1024
S = 2048
B = 16
NCORES = 8
BL = B // NCORES  # local batches per core
P = 128
KT = H // P  # h_in tiles
MT = H // P  # h_out tiles
NF = 512  # s-tile width for matmuls
SH = 2  # s halves (1024 each) per enc DMA group
SI = S // NF  # s512 tiles per batch

F32 = mybir.dt.float32
F32R = mybir.dt.float32r
I32 = mybir.dt.int32
AF = mybir.ActivationFunctionType
AX = mybir.AxisListType


def _build_program():
    nc = bacc.Bacc("TRN2", target_bir_lowering=False, debug=False, num_devices=NCORES)

    encT = nc.dram_tensor("encT", [BL, H, S], F32R, kind="ExternalInput").ap()
    weM = nc.dram_tensor("weM", [MT, H, P], F32R, kind="ExternalInput").ap()
    whT = nc.dram_tensor("whT", [H, H], F32R, kind="ExternalInput").ap()
    hid16 = nc.dram_tensor("hid16", [P, KT * BL], F32R, kind="ExternalInput").ap()
    bias8 = nc.dram_tensor("bias8", [P, MT], F32, kind="ExternalInput").ap()
    score8 = nc.dram_tensor("score8", [P, MT], F32R, kind="ExternalInput").ap()
    mask = nc.dram_tensor("mask", [BL, S], I32, kind="ExternalInput").ap()
    out = nc.dram_tensor("out", [BL, S], F32, kind="ExternalOutput").ap()

    with tile.TileContext(nc) as tc:
        with (
            tc.tile_pool(name="consts", bufs=1) as cpool,
            tc.tile_pool(name="weights", bufs=1) as wpool,
            tc.tile_pool(name="enc", bufs=2) as epool,
            tc.tile_pool(name="proj", bufs=6) as ppool,
            tc.tile_pool(name="soft", bufs=1) as spool,
            tc.tile_pool(name="mm", bufs=5, space="PSUM") as mmpool,
            tc.tile_pool(name="esc", bufs=2, space="PSUM") as epsum,
            tc.tile_pool(name="hp", bufs=1, space="PSUM") as hpsum,
        ):
            # --- constants -------------------------------------------------
            hid_sb = cpool.tile([P, KT * BL], F32R, tag="hid")
            nc.sync.dma_start(hid_sb[:], hid16[:])
            bias8_sb = cpool.tile([P, MT], F32, tag="bias8")
            nc.sync.dma_start(bias8_sb[:], bias8[:])
            score8_sb = cpool.tile([P, MT], F32R, tag="score8")
            nc.sync.dma_start(score8_sb[:], score8[:])

            # per-b mask rows on partition 0 (compute engines need aligned
            # start partitions, so don't slice partition 1 of a [2, S] tile)
            mask_sb = []
            for b in range(BL):
                mask_i = cpool.tile([1, S], I32, tag=f"mask_i{b}", name=f"mask_i{b}")
                nc.sync.dma_start(mask_i[:], mask[b : b + 1, :])
                mask_sb.append(mask_i)

            # --- weights, m-major: we_sb[m][:, k*P:(k+1)*P] is the lhsT
            # chunk for (k, m). DMA order is chosen so the first m-group's
            # inputs (we m=0, 0.5 MB + enc group 0, 4 MB) land first; the
            # remaining m chunks stream in one step ahead of the m-loop.
            we_sb = []
            for m in range(MT):
                t = wpool.tile([P, H], F32R, tag=f"wem{m}", name=f"wem{m}")
                nc.sync.dma_start(
                    t[:].rearrange("p (k j) -> p k j", k=KT),
                    weM[m].rearrange("(k p) j -> p k j", p=P),
                )
                we_sb.append(t)
                if m == 0:
                    enc_g0 = []
                    for k in range(KT):
                        e = epool.tile(
                            [P, S // SH], F32R, tag=f"enc{k}", name=f"enc_g0_{k}"
                        )
                        nc.sync.dma_start(
                            e[:], encT[0, k * P : (k + 1) * P, 0 : S // SH]
                        )
                        enc_g0.append(e)
            wh_sb = []
            for k in range(KT):
                t = wpool.tile([P, H], F32R, tag=f"wh{k}")
                nc.sync.dma_start(t[:], whT[k * P : (k + 1) * P, :])
                wh_sb.append(t)

            # --- hidden projection: hid_proj^T[m-tile] is [128, BL] -------
            hidp_ps = hpsum.tile([P, MT * BL], F32, tag="hidp")
            for m in range(MT):
                o = hidp_ps[:, m * BL : (m + 1) * BL]
                for k in range(KT):
                    nc.tensor.matmul(
                        o,
                        lhsT=wh_sb[k][:, m * P : (m + 1) * P],
                        rhs=hid_sb[:, k * BL : (k + 1) * BL],
                        start=(k == 0),
                        stop=(k == KT - 1),
                    )
            # bias_sb[:, m*BL + b] = attn_b[m-tile] + hid_proj[m-tile, b]
            bias_sb = cpool.tile([P, MT * BL], F32, tag="bias_mb")
            for m in range(MT):
                nc.scalar.activation(
                    bias_sb[:, m * BL : (m + 1) * BL],
                    hidp_ps[:, m * BL : (m + 1) * BL],
                    AF.Identity,
                    bias=bias8_sb[:, m : m + 1],
                )

            # --- softmax accumulators -------------------------------------
            exp_sb = [spool.tile([1, S], F32, tag=f"exp{b}", name=f"exp{b}") for b in range(BL)]
            sums_sb = [spool.tile([1, SI], F32, tag=f"sums{b}", name=f"sums{b}") for b in range(BL)]

            # --- main loop -------------------------------------------------
            for b in range(BL):
                for sh in range(SH):
                    if b == 0 and sh == 0:
                        enc_t = enc_g0
                    else:
                        enc_t = []
                        for k in range(KT):
                            t = epool.tile(
                                [P, S // SH],
                                F32R,
                                tag=f"enc{k}",
                                name=f"enc_{b}_{sh}_{k}",
                            )
                            nc.sync.dma_start(
                                t[:],
                                encT[
                                    b,
                                    k * P : (k + 1) * P,
                                    sh * (S // SH) : (sh + 1) * (S // SH),
                                ],
                            )
                            enc_t.append(t)
                    for sj in range(S // SH // NF):
                        si = sh * (S // SH // NF) + sj
                        es_ps = epsum.tile([1, NF], F32, tag="escore")
                        for m in range(MT):
                            mm_ps = mmpool.tile([P, NF], F32, tag="mm")
                            for k in range(KT):
                                nc.tensor.matmul(
                                    mm_ps[:],
                                    lhsT=we_sb[m][:, k * P : (k + 1) * P],
                                    rhs=enc_t[k][:, sj * NF : (sj + 1) * NF],
                                    start=(k == 0),
                                    stop=(k == KT - 1),
                                )
                            proj = ppool.tile([P, NF], F32R, tag="proj")
                            nc.scalar.activation(
                                proj[:],
                                mm_ps[:],
                                AF.Tanh,
                                bias=bias_sb[:, m * BL + b : m * BL + b + 1],
                            )
                            nc.tensor.matmul(
                                es_ps[:],
                                lhsT=score8_sb[:, m : m + 1],
                                rhs=proj[:],
                                start=(m == 0),
                                stop=(m == MT - 1),
                            )
                        masked = ppool.tile([1, NF], F32, tag="masked")
                        nc.vector.scalar_tensor_tensor(
                            masked[:],
                            mask_sb[b][0:1, si * NF : (si + 1) * NF],
                            -1.0e12,
                            es_ps[:],
                            op0=mybir.AluOpType.mult,
                            op1=mybir.AluOpType.add,
                        )
                        nc.scalar.activation(
                            exp_sb[b][0:1, si * NF : (si + 1) * NF],
                            masked[:],
                            AF.Exp,
                            accum_out=sums_sb[b][0:1, si : si + 1],
                        )

            # --- normalize + store ----------------------------------------
            for b in range(BL):
                tot = spool.tile([1, 1], F32, tag=f"tot{b}")
                nc.vector.reduce_sum(tot[:], sums_sb[b][:], axis=AX.X)
                rec = spool.tile([1, 1], F32, tag=f"rec{b}")
                nc.vector.reciprocal(rec[:], tot[:])
                nc.vector.tensor_scalar_mul(exp_sb[b][:], exp_sb[b][:], rec[:])
                nc.sync.dma_start(out[b : b + 1, :], exp_sb[b][:])

    nc.compile()
    return nc


_NC = None


def _get_program():
    global _NC
    if _NC is None:
        _NC = _build_program()
    return _NC


def make_in_maps(hidden, encoder_outputs, seq_mask, attn_w, attn_b, score_w):
    """Slice/relayout the full inputs into the 8 per-core input maps."""
    hidden = np.asarray(hidden, dtype=np.float32)
    encoder_outputs = np.asarray(encoder_outputs, dtype=np.float32)
    seq_mask = np.ascontiguousarray(np.asarray(seq_mask, dtype=np.int32))
    attn_w = np.asarray(attn_w, dtype=np.float32)
    attn_b = np.asarray(attn_b, dtype=np.float32)
    score_w = np.asarray(score_w, dtype=np.float32)

    weT = attn_w[:, H:].T  # [h_in, h_out]
    # m-major: weM[m, ki, j] = weT[ki, m*P + j]
    weM = np.ascontiguousarray(weT.reshape(H, MT, P).transpose(1, 0, 2))
    whT = np.ascontiguousarray(attn_w[:, :H].T)  # [h_in, h_out]
    bias8 = np.ascontiguousarray(attn_b.reshape(MT, P).T)  # [128, MT]
    score8 = np.ascontiguousarray(score_w[0].reshape(MT, P).T)  # [128, MT]
    encT = encoder_outputs.transpose(1, 2, 0)  # [B, H, S]
    hidT = hidden[0].T  # [H, B]

    in_maps = []
    for c in range(NCORES):
        bsl = slice(c * BL, (c + 1) * BL)
        hid16 = np.ascontiguousarray(
            hidT[:, bsl].reshape(KT, P, BL).transpose(1, 0, 2).reshape(P, KT * BL)
        )
        in_maps.append(
            {
                "encT": np.ascontiguousarray(encT[bsl]),
                "weM": weM,
                "whT": whT,
                "hid16": hid16,
                "bias8": bias8,
                "score8": score8,
                "mask": np.ascontiguousarray(seq_mask[bsl]),
            }
        )
    return in_maps


def gather_output(results):
    outs = np.concatenate([results[c]["out"] for c in range(NCORES)], axis=0)
    return np.ascontiguousarray(outs[:, None, :].astype(np.float32))


def kernel(hidden, encoder_outputs, seq_mask, attn_w, attn_b, score_w):
    nc = _get_program()
    in_maps = make_in_maps(
        hidden, encoder_outputs, seq_mask, attn_w, attn_b, score_w
    )
    res = run_bass_kernel_spmd(nc, in_maps, list(range(NCORES)))
    return gather_output(res.results)


# revision 13
# speedup vs baseline: 1.0487x; 1.0487x over previous
"""Bahdanau-style attention kernel for Trainium2, data-parallel over batch.

Math (per (s, b)):
    pre[s,b,:]  = We @ enc[s,b,:] + Wh @ hidden[b,:] + attn_b      (H outputs)
    energies    = score_w . tanh(pre)                               -> [S, B]
    out         = softmax over S of (energies masked to -1e12)      -> [B, 1, S]

Sharding: B=16 batches split 2-per-core over 8 NeuronCores; weights are
replicated; no collectives. Each core runs one identical Bass program on
its own input slice.

Per-core layout: the main GEMM computes proj^T tiles [h_out=128, s=512]
with We^T chunks stationary (fp32r -> full PE rate at free dim 512) and
enc^T tiles moving; tanh+bias (attn_b + Wh@hidden, both per-partition
constants for a fixed b) is fused into one ScalarE activation reading
PSUM; the score contraction over h_out is a second accumulating matmul
(score column stationary), so energies for 512 positions land in PSUM as
[1, 512]; masking adds -1e12*mask and exp+sum are fused via activation
accum_out (max-subtraction is skipped: energies are O(1) bounded by
|score_w|_1, so exp never overflows, and exp(-1e12) == 0 exactly).
"""

import sys

for _p in ("/opt/trn_rl_repo", "/opt/pypackages"):
    if _p not in sys.path:
        sys.path.append(_p)

import numpy as np

from concourse import bacc, mybir, tile
from concourse.bass_utils import run_bass_kernel_spmd

H = 1024
S = 2048
B = 16
NCORES = 8
BL = B // NCORES  # local batches per core
P = 128
KT = H // P  # h_in tiles
MT = H // P  # h_out tiles
NF = 512  # s-tile width for matmuls
SH = 2  # s halves (1024 each) per enc DMA group
SI = S // NF  # s512 tiles per batch

F32 = mybir.dt.float32
F32R = mybir.dt.float32r
I32 = mybir.dt.int32
AF = mybir.ActivationFunctionType
AX = mybir.AxisListType


def _build_program():
    nc = bacc.Bacc("TRN2", target_bir_lowering=False, debug=False, num_devices=NCORES)

    encT = nc.dram_tensor("encT", [BL, H, S], F32R, kind="ExternalInput").ap()
    weT = nc.dram_tensor("weT", [H, H], F32R, kind="ExternalInput").ap()
    whT = nc.dram_tensor("whT", [H, H], F32R, kind="ExternalInput").ap()
    hid16 = nc.dram_tensor("hid16", [P, KT * BL], F32R, kind="ExternalInput").ap()
    bias8 = nc.dram_tensor("bias8", [P, MT], F32, kind="ExternalInput").ap()
    score8 = nc.dram_tensor("score8", [P, MT], F32R, kind="ExternalInput").ap()
    mask = nc.dram_tensor("mask", [BL, S], I32, kind="ExternalInput").ap()
    out = nc.dram_tensor("out", [BL, S], F32, kind="ExternalOutput").ap()

    with tile.TileContext(nc) as tc:
        with (
            tc.tile_pool(name="consts", bufs=1) as cpool,
            tc.tile_pool(name="weights", bufs=1) as wpool,
            tc.tile_pool(name="enc", bufs=2) as epool,
            tc.tile_pool(name="proj", bufs=6) as ppool,
            tc.tile_pool(name="soft", bufs=1) as spool,
            tc.tile_pool(name="mm", bufs=4, space="PSUM") as mmpool,
            tc.tile_pool(name="esc", bufs=3, space="PSUM") as epsum,
            tc.tile_pool(name="hp", bufs=1, space="PSUM") as hpsum,
        ):
            # --- constants -------------------------------------------------
            hid_sb = cpool.tile([P, KT * BL], F32R, tag="hid")
            nc.sync.dma_start(hid_sb[:], hid16[:])
            bias8_sb = cpool.tile([P, MT], F32, tag="bias8")
            nc.sync.dma_start(bias8_sb[:], bias8[:])
            score8_sb = cpool.tile([P, MT], F32R, tag="score8")
            nc.sync.dma_start(score8_sb[:], score8[:])

            # per-b mask rows on partition 0 (compute engines need aligned
            # start partitions, so don't slice partition 1 of a [2, S] tile)
            mask_sb = []
            for b in range(BL):
                mask_i = cpool.tile([1, S], I32, tag=f"mask_i{b}", name=f"mask_i{b}")
                nc.sync.dma_start(mask_i[:], mask[b : b + 1, :])
                mask_sb.append(mask_i)

            # --- weights first: a dense k-loop needs all of We before any
            # m-group completes. Starting compute during the weight DMA was
            # measured (twice) to DMA-starve the PE into HAM half-clock.
            we_sb = []
            for k in range(KT):
                t = wpool.tile([P, H], F32R, tag=f"we{k}")
                nc.sync.dma_start(t[:], weT[k * P : (k + 1) * P, :])
                we_sb.append(t)
            wh_sb = []
            for k in range(KT):
                t = wpool.tile([P, H], F32R, tag=f"wh{k}")
                nc.sync.dma_start(t[:], whT[k * P : (k + 1) * P, :])
                wh_sb.append(t)

            # --- hidden projection: hid_proj^T[m-tile] is [128, BL] -------
            hidp_ps = hpsum.tile([P, MT * BL], F32, tag="hidp")
            for m in range(MT):
                o = hidp_ps[:, m * BL : (m + 1) * BL]
                for k in range(KT):
                    nc.tensor.matmul(
                        o,
                        lhsT=wh_sb[k][:, m * P : (m + 1) * P],
                        rhs=hid_sb[:, k * BL : (k + 1) * BL],
                        start=(k == 0),
                        stop=(k == KT - 1),
                    )
            # bias_sb[:, m*BL + b] = attn_b[m-tile] + hid_proj[m-tile, b]
            bias_sb = cpool.tile([P, MT * BL], F32, tag="bias_mb")
            for m in range(MT):
                nc.scalar.activation(
                    bias_sb[:, m * BL : (m + 1) * BL],
                    hidp_ps[:, m * BL : (m + 1) * BL],
                    AF.Identity,
                    bias=bias8_sb[:, m : m + 1],
                )

            # --- softmax accumulators -------------------------------------
            exp_sb = [spool.tile([1, S], F32, tag=f"exp{b}", name=f"exp{b}") for b in range(BL)]
            sums_sb = [spool.tile([1, SI], F32, tag=f"sums{b}", name=f"sums{b}") for b in range(BL)]

            # --- main loop -------------------------------------------------
            for b in range(BL):
                for sh in range(SH):
                    enc_t = []
                    for k in range(KT):
                        t = epool.tile(
                            [P, S // SH], F32R, tag=f"enc{k}", name=f"enc_{b}_{sh}_{k}"
                        )
                        nc.sync.dma_start(
                            t[:],
                            encT[
                                b,
                                k * P : (k + 1) * P,
                                sh * (S // SH) : (sh + 1) * (S // SH),
                            ],
                        )
                        enc_t.append(t)
                    nsj = S // SH // NF
                    es_ps = [
                        epsum.tile([1, NF], F32, tag="escore", name=f"es_{b}_{sh}_{sj}")
                        for sj in range(nsj)
                    ]
                    for m in range(MT):
                        mm_ps = [
                            mmpool.tile([P, NF], F32, tag="mm", name=f"mm_{m}_{sj}")
                            for sj in range(nsj)
                        ]
                        for k in range(KT):
                            for sj in range(nsj):
                                nc.tensor.matmul(
                                    mm_ps[sj][:],
                                    lhsT=we_sb[k][:, m * P : (m + 1) * P],
                                    rhs=enc_t[k][:, sj * NF : (sj + 1) * NF],
                                    start=(k == 0),
                                    stop=(k == KT - 1),
                                )
                        for sj in range(nsj):
                            proj = ppool.tile([P, NF], F32R, tag="proj")
                            nc.scalar.activation(
                                proj[:],
                                mm_ps[sj][:],
                                AF.Tanh,
                                bias=bias_sb[:, m * BL + b : m * BL + b + 1],
                            )
                            nc.tensor.matmul(
                                es_ps[sj][:],
                                lhsT=score8_sb[:, m : m + 1],
                                rhs=proj[:],
                                start=(m == 0),
                                stop=(m == MT - 1),
                            )
                    for sj in range(nsj):
                        si = sh * nsj + sj
                        masked = ppool.tile([1, NF], F32, tag="masked")
                        nc.vector.scalar_tensor_tensor(
                            masked[:],
                            mask_sb[b][0:1, si * NF : (si + 1) * NF],
                            -1.0e12,
                            es_ps[sj][:],
                            op0=mybir.AluOpType.mult,
                            op1=mybir.AluOpType.add,
                        )
                        nc.scalar.activation(
                            exp_sb[b][0:1, si * NF : (si + 1) * NF],
                            masked[:],
                            AF.Exp,
                            accum_out=sums_sb[b][0:1, si : si + 1],
                        )

            # --- normalize + store ----------------------------------------
            for b in range(BL):
                tot = spool.tile([1, 1], F32, tag=f"tot{b}")
                nc.vector.reduce_sum(tot[:], sums_sb[b][:], axis=AX.X)
                rec = spool.tile([1, 1], F32, tag=f"rec{b}")
                nc.vector.reciprocal(rec[:], tot[:])
                nc.vector.tensor_scalar_mul(exp_sb[b][:], exp_sb[b][:], rec[:])
                nc.sync.dma_start(out[b : b + 1, :], exp_sb[b][:])

    nc.compile()
    return nc


_NC = None


def _get_program():
    global _NC
    if _NC is None:
        _NC = _build_program()
    return _NC


def make_in_maps(hidden, encoder_outputs, seq_mask, attn_w, attn_b, score_w):
    """Slice/relayout the full inputs into the 8 per-core input maps."""
    hidden = np.asarray(hidden, dtype=np.float32)
    encoder_outputs = np.asarray(encoder_outputs, dtype=np.float32)
    seq_mask = np.ascontiguousarray(np.asarray(seq_mask, dtype=np.int32))
    attn_w = np.asarray(attn_w, dtype=np.float32)
    attn_b = np.asarray(attn_b, dtype=np.float32)
    score_w = np.asarray(score_w, dtype=np.float32)

    weT = np.ascontiguousarray(attn_w[:, H:].T)  # [h_in, h_out]
    whT = np.ascontiguousarray(attn_w[:, :H].T)  # [h_in, h_out]
    bias8 = np.ascontiguousarray(attn_b.reshape(MT, P).T)  # [128, MT]
    score8 = np.ascontiguousarray(score_w[0].reshape(MT, P).T)  # [128, MT]
    encT = encoder_outputs.transpose(1, 2, 0)  # [B, H, S]
    hidT = hidden[0].T  # [H, B]

    in_maps = []
    for c in range(NCORES):
        bsl = slice(c * BL, (c + 1) * BL)
        hid16 = np.ascontiguousarray(
            hidT[:, bsl].reshape(KT, P, BL).transpose(1, 0, 2).reshape(P, KT * BL)
        )
        in_maps.append(
            {
                "encT": np.ascontiguousarray(encT[bsl]),
                "weT": weT,
                "whT": whT,
                "hid16": hid16,
                "bias8": bias8,
                "score8": score8,
                "mask": np.ascontiguousarray(seq_mask[bsl]),
            }
        )
    return in_maps


def gather_output(results):
    outs = np.concatenate([results[c]["out"] for c in range(NCORES)], axis=0)
    return np.ascontiguousarray(outs[:, None, :].astype(np.float32))


def kernel(hidden, encoder_outputs, seq_mask, attn_w, attn_b, score_w):
    nc = _get_program()
    in_maps = make_in_maps(
        hidden, encoder_outputs, seq_mask, attn_w, attn_b, score_w
    )
    res = run_bass_kernel_spmd(nc, in_maps, list(range(NCORES)))
    return gather_output(res.results)
